# revision 2
# baseline (speedup 1.0000x reference)
"""Savitzky-Golay noise-reduction filter (window=11, poly=3) on Trainium2.

Input x: (64, 16, 65536) fp32. Output: same shape, savgol filtered along the
last axis with scipy mode='interp' edge handling (polynomial fit on the first/
last WINDOW samples).

Strategy (pure data parallel, 8 cores, 128 rows of length 65536 per core):
  - Overlapping 128-sample input windows (stride 118) are PE-transposed so
    time lies on partitions, then one fp32 matmul with a banded weight matrix
    computes 118 (123 for edge windows) outputs per window; edge-polynomial
    rows are folded into the first/last windows' weight matrices.
  - PE transpose-back returns natural layout; results are staged in SBUF and
    stored with large contiguous DMAs.
"""

from contextlib import ExitStack

import numpy as np

WINDOW = 11
POLY = 3
HALF = WINDOW // 2  # 5
P = 128
L = 65536
N_CORES = 8
ROWS_TOTAL = 1024  # 64*16
ROWS_PER_CORE = ROWS_TOTAL // N_CORES  # 128
STRIDE = P - (WINDOW - 1)  # 118
EDGE_W = P - HALF  # 123: outputs of first/last window


def _savgol_matrices():
    pos = np.arange(-HALF, HALF + 1, dtype=np.float64)
    A = pos[:, None] ** np.arange(POLY + 1)[None, :]
    c = np.linalg.pinv(A)[0]  # [W] central taps
    V = np.arange(WINDOW, dtype=np.float64)[:, None] ** np.arange(POLY + 1)[None, :]
    T = np.arange(HALF, dtype=np.float64)[:, None] ** np.arange(POLY + 1)[None, :]
    E = T @ np.linalg.pinv(V)  # [HALF, W]
    return c, E


def _build_weights():
    """W matrices [128, 128] (stationary lhsT: W[q, p] multiplies input q for
    output p).  Returns (W_first, W_mid, W_last) float32."""
    c64, E64 = _savgol_matrices()
    c = c64.astype(np.float32)
    E = E64.astype(np.float32)

    w_mid = np.zeros((P, P), np.float32)
    for p in range(STRIDE):  # output p -> y[base + 5 + p] = sum_k c_k x[base+p+k]
        for k in range(WINDOW):
            w_mid[p + k, p] = c[k]

    w_first = np.zeros((P, P), np.float32)
    for p in range(HALF):  # y[p] = sum_w E[p, w] x[w]
        for w in range(WINDOW):
            w_first[w, p] = E[p, w]
    for p in range(HALF, EDGE_W):  # y[p] = sum_k c_k x[p - 5 + k]
        for k in range(WINDOW):
            w_first[p - HALF + k, p] = c[k]

    w_last = np.zeros((P, P), np.float32)
    for p in range(STRIDE):  # y[L-123+p] = sum_k c_k x[base + p + k]
        for k in range(WINDOW):
            w_last[p + k, p] = c[k]
    for h in range(HALF):  # y[L-5+h] = sum_w E[4-h, w] x[L-1-w] = x[base+127-w]
        for w in range(WINDOW):
            w_last[127 - w, STRIDE + h] = E[HALF - 1 - h, w]

    return w_first, w_mid, w_last


def _windows():
    """List of (in_off, out_off, out_width, wtype) per window; wtype in
    {'first','mid','last'}."""
    wins = [(0, 0, EDGE_W, "first")]
    j = 1
    while True:
        in_off = STRIDE * j
        out_off = in_off + HALF
        if out_off + STRIDE >= L - HALF:
            break
        wins.append((in_off, out_off, STRIDE, "mid"))
        j += 1
    wins.append((L - P, L - EDGE_W, EDGE_W, "last"))
    return wins


def reference_rows(x):
    """Numpy reference for [rows, L] (mirrors the jax reference)."""
    c64, E64 = _savgol_matrices()
    c = c64.astype(np.float32)
    E = E64.astype(np.float32)
    R, Lx = x.shape
    out = np.empty_like(x)
    # interior via correlation
    from numpy.lib.stride_tricks import sliding_window_view

    sw = sliding_window_view(x, WINDOW, axis=1)  # [R, L-10, 11]
    out[:, HALF : Lx - HALF] = np.einsum("rlk,k->rl", sw, c, optimize=True).astype(
        np.float32
    )
    out[:, :HALF] = x[:, :WINDOW] @ E.T
    out[:, Lx - HALF :] = (x[:, ::-1][:, :WINDOW] @ E.T)[:, ::-1]
    return out


def simulate_host(x):
    """Pure-numpy simulation of the windowed scheme, to validate W matrices."""
    w_first, w_mid, w_last = _build_weights()
    wmap = {"first": w_first, "mid": w_mid, "last": w_last}
    R, Lx = x.shape
    out = np.zeros_like(x)
    for in_off, out_off, width, t in _windows():
        xw = x[:, in_off : in_off + P]  # [R, 128]
        yw = xw @ wmap[t]  # [R, 128]
        out[:, out_off : out_off + width] = yw[:, :width]
    return out


# ---------------------------------------------------------------------------
# Bass kernel
# ---------------------------------------------------------------------------

IO_BATCH = 32  # windows per DMA in/out batch
CONV_GROUP = 4  # windows per conv matmul (moving dim 4*128=512)

_NC_CACHE = None


def _build_nc(reps: int = 1, n_batches: int | None = None, ablate: frozenset = frozenset()):
    import concourse.tile as tile
    from concourse import bacc, mybir
    from concourse.masks import make_identity

    nc = bacc.Bacc(
        "TRN2",
        target_bir_lowering=False,
        debug=False,
        enable_asserts=False,
        num_devices=N_CORES,
    )
    x = nc.dram_tensor("x", [P, L], mybir.dt.float32, kind="ExternalInput").ap()
    wf = nc.dram_tensor("w_first", [P, P], mybir.dt.float32, kind="ExternalInput").ap()
    wm = nc.dram_tensor("w_mid", [P, P], mybir.dt.float32, kind="ExternalInput").ap()
    wl = nc.dram_tensor("w_last", [P, P], mybir.dt.float32, kind="ExternalInput").ap()
    y = nc.dram_tensor("y", [P, L], mybir.dt.float32, kind="ExternalOutput").ap()

    wins = _windows()
    batches = [wins[i : i + IO_BATCH] for i in range(0, len(wins), IO_BATCH)]
    if n_batches is not None:
        batches = batches[:n_batches]

    f32 = mybir.dt.float32

    with tile.TileContext(nc) as tc:
        with ExitStack() as ctx:
            consts = ctx.enter_context(tc.tile_pool(name="consts", bufs=1))
            in_pool = ctx.enter_context(tc.tile_pool(name="inp", bufs=3))
            out_pool = ctx.enter_context(tc.tile_pool(name="outp", bufs=3))
            xt_pool = ctx.enter_context(tc.tile_pool(name="xt", bufs=4))
            ps_t = ctx.enter_context(tc.tile_pool(name="ps_t", bufs=4, space="PSUM"))
            ps_c = ctx.enter_context(tc.tile_pool(name="ps_c", bufs=4, space="PSUM"))

            ident = consts.tile([P, P], f32, tag="ident")
            make_identity(nc, ident[:])
            wt = {}
            for name, ap in (("first", wf), ("mid", wm), ("last", wl)):
                t = consts.tile([P, P], f32, tag=f"w_{name}")
                nc.sync.dma_start(t[:], ap)
                wt[name] = t

            copy_cnt = 0  # distribute copies ACT-heavy (DVE pays a DRAIN)
            act_share = 5  # out of 8 copies go to ACT

            def copy(dst, src):
                nonlocal copy_cnt
                if copy_cnt % 8 < act_share:
                    nc.scalar.copy(dst, src)
                else:
                    nc.vector.tensor_copy(dst, src)
                copy_cnt += 1

            for _rep in range(reps):
              for batch in batches:
                in_base = batch[0][0]
                in_span = batch[-1][0] + P - in_base
                out_base = batch[0][1]
                out_span = batch[-1][1] + batch[-1][2] - out_base

                xin = in_pool.tile([P, IO_BATCH * STRIDE + 256], f32, tag="xin")
                nc.sync.dma_start(
                    xin[:, :in_span], x[:, in_base : in_base + in_span]
                )
                stag = out_pool.tile([P, IO_BATCH * STRIDE + 256], f32, tag="stag")

                # group windows by consecutive same-type runs of <= CONV_GROUP
                groups = []
                cur = []
                for w in batch:
                    if cur and (w[3] != cur[0][3] or len(cur) == CONV_GROUP):
                        groups.append(cur)
                        cur = []
                    cur.append(w)
                groups.append(cur)

                for grp in groups:
                    g = len(grp)
                    wtype = grp[0][3]
                    # 1) PE transpose each window into one PSUM bank
                    pt = ps_t.tile([P, 512], f32, tag="pt")
                    for s, (in_off, _, _, _) in enumerate(grp):
                        o = in_off - in_base
                        nc.tensor.transpose(
                            pt[:, s * P : (s + 1) * P],
                            xin[:, o : o + P],
                            ident[:],
                        )
                    # 2) copy PSUM -> SBUF
                    xt = xt_pool.tile([P, 512], f32, tag="xt")
                    copy(xt[:, : g * P], pt[:, : g * P])
                    # 3) conv matmul per window, stationary = transposed window,
                    #    moving = W  ->  output lands in NATURAL layout
                    width = grp[0][2]
                    pc = ps_c.tile([P, 512], f32, tag="pc")
                    for s in range(g):
                        nc.tensor.matmul(
                            pc[:, s * P : s * P + width],
                            xt[:, s * P : (s + 1) * P],
                            wt[wtype][:, :width],
                            start=True,
                            stop=True,
                        )
                    # 4) copy valid columns into the output staging buffer
                    soff = grp[0][1] - out_base
                    src = pc[:].rearrange("p (s t) -> p s t", s=4)[
                        :, :g, :width
                    ]
                    dst = stag[:, soff : soff + g * width].rearrange(
                        "p (s t) -> p s t", s=g
                    )
                    copy(dst, src)

                nc.sync.dma_start(
                    y[:, out_base : out_base + out_span], stag[:, :out_span]
                )

    nc.compile()
    return nc


def _get_nc():
    global _NC_CACHE
    if _NC_CACHE is None:
        _NC_CACHE = _build_nc()
    return _NC_CACHE


def _in_maps(x: np.ndarray) -> list[dict]:
    w_first, w_mid, w_last = _build_weights()
    xr = np.ascontiguousarray(x.reshape(ROWS_TOTAL, L))
    return [
        {
            "x": xr[i * ROWS_PER_CORE : (i + 1) * ROWS_PER_CORE],
            "w_first": w_first,
            "w_mid": w_mid,
            "w_last": w_last,
        }
        for i in range(N_CORES)
    ]


def kernel(x: np.ndarray) -> np.ndarray:
    from concourse.bass_utils import run_bass_kernel_spmd

    assert x.shape == (64, 16, L) and x.dtype == np.float32
    nc = _get_nc()
    in_maps = _in_maps(x)
    res = run_bass_kernel_spmd(nc, in_maps, core_ids=list(range(N_CORES)))
    out = np.concatenate([r["y"] for r in res.results], axis=0)
    return out.reshape(64, 16, L)


if __name__ == "__main__":
    # host-side validation of the window scheme
    rng = np.random.default_rng(0)
    xt = rng.standard_normal((4, L)).astype(np.float32)
    ref = reference_rows(xt)
    sim = simulate_host(xt)
    err = np.abs(sim - ref).max()
    rel = err / np.abs(ref).max()
    print(f"host sim vs ref: max abs {err:.3e}  rel {rel:.3e}")
    print("n windows:", len(_windows()))



# revision 3
# speedup vs baseline: 1.0210x; 1.0210x over previous
"""Savitzky-Golay filter (window=11, poly=3) on Trainium2.

Layout strategy (per core: 128 rows x 65536 cols, pure data parallel):
  - DMA in plain contiguous chunks (the memory roofline).
  - DVE 32x32 block-transpose puts time-within-32-blocks on partitions.
  - PE computes the FIR as TWO fat weight-stationary matmuls per 512-col
    PSUM bank: banded block-diagonal tap matrices W_in / W_next contract
    over the 32 time positions (4 row-blocks in parallel); the +32-shifted
    rhs covers taps that spill into the next time block. Output frame is
    shifted +5 so each output block needs only 2 input blocks.
  - DVE block-transposes PSUM back to natural layout (replaces copies).
  - DMA out plain contiguous chunks.
  - scipy mode='interp' edges via two tiny extra matmuls (E-fit matrices
    folded into 128x128 stationaries) merged into the first/last chunk.
"""

from contextlib import ExitStack

import numpy as np

WINDOW = 11
POLY = 3
HALF = WINDOW // 2  # 5
P = 128
L = 65536
N_CORES = 8
ROWS_TOTAL = 1024
ROWS_PER_CORE = ROWS_TOTAL // N_CORES  # 128
B = 32  # DVE stream-transpose block size
BW = 512  # psum bank width (fp32 cols)
PSW = 1024  # psum tile width (2 banks) per out-transpose
CH = 4096  # main chunk width (2.1MB DMAs: needed for full ~380GB/s DMA rate)
CH_EDGE = 2048  # first/last chunks run half-width to cut fill/drain latency


def _savgol_matrices():
    pos = np.arange(-HALF, HALF + 1, dtype=np.float64)
    A = pos[:, None] ** np.arange(POLY + 1)[None, :]
    c = np.linalg.pinv(A)[0]  # [W] central taps
    V = np.arange(WINDOW, dtype=np.float64)[:, None] ** np.arange(POLY + 1)[None, :]
    T = np.arange(HALF, dtype=np.float64)[:, None] ** np.arange(POLY + 1)[None, :]
    E = T @ np.linalg.pinv(V)  # [HALF, W]
    return c, E


def _build_weights():
    """Stationary lhsT matrices [128, 128] fp32: out[p,f] = sum_q W[q,p] rhs[q,f].

    Block-diagonal over 4 row-blocks (a).  Within a block (i = time-in-block
    of rhs, o = output-time-in-block, +5 frame shift):
      W_in[i, o]   = c[i - o]        (0 <= i-o <= 10)
      W_next[i, o] = c[i + 32 - o]   (taps spilling into the next block)
      W_ef[i, h]   = E[h, i]         (left edge: y[:, h], h < 5, from x[0:11])
      W_el[i, h]   = E[4-h, 31-i]    (right edge: y[:, L-5+h] from x[L-11:L])
    """
    c64, E64 = _savgol_matrices()
    c = c64.astype(np.float32)
    E = E64.astype(np.float32)

    w_in32 = np.zeros((B, B), np.float32)
    w_nx32 = np.zeros((B, B), np.float32)
    for o in range(B):
        for k in range(WINDOW):
            m = o + k
            if m < B:
                w_in32[m, o] = c[k]
            else:
                w_nx32[m - B, o] = c[k]
    w_ef32 = np.zeros((B, B), np.float32)
    for h in range(HALF):
        for w in range(WINDOW):
            w_ef32[w, h] = E[h, w]
    w_el32 = np.zeros((B, B), np.float32)
    for h in range(HALF):
        for w in range(WINDOW):
            w_el32[31 - w, h] = E[HALF - 1 - h, w]

    def blockdiag(w32):
        W = np.zeros((P, P), np.float32)
        for a in range(P // B):
            W[a * B : (a + 1) * B, a * B : (a + 1) * B] = w32
        return W

    return blockdiag(w_in32), blockdiag(w_nx32), blockdiag(w_ef32), blockdiag(w_el32)


def _block_t(m):
    """32x32 block transpose of [128, F] (F % 32 == 0)."""
    p, f = m.shape
    v = m.reshape(p // B, B, f // B, B)
    return np.ascontiguousarray(v.transpose(0, 3, 2, 1)).reshape(p, f)


def reference_rows(x):
    c64, E64 = _savgol_matrices()
    c = c64.astype(np.float32)
    E = E64.astype(np.float32)
    R, Lx = x.shape
    out = np.empty_like(x)
    from numpy.lib.stride_tricks import sliding_window_view

    sw = sliding_window_view(x, WINDOW, axis=1)
    out[:, HALF : Lx - HALF] = np.einsum("rlk,k->rl", sw, c, optimize=True).astype(
        np.float32
    )
    out[:, :HALF] = x[:, :WINDOW] @ E.T
    out[:, Lx - HALF :] = (x[:, ::-1][:, :WINDOW] @ E.T)[:, ::-1]
    return out


def simulate_host(x, ch=CH):
    """Numpy simulation of the on-device scheme, to validate W + indexing."""
    w_in, w_nx, w_ef, w_el = _build_weights()
    R, Lx = x.shape
    assert R == P and Lx % ch == 0
    out = np.zeros_like(x)
    n_chunks = Lx // ch
    for n in range(n_chunks):
        base = n * ch
        xin = np.zeros((P, ch + B), np.float32)
        valid = min(ch + B, Lx - base)
        xin[:, :valid] = x[:, base : base + valid]
        btx = _block_t(xin)
        for k in range(ch // BW):
            rhs1 = btx[:, k * BW : k * BW + BW]
            rhs2 = btx[:, k * BW + B : k * BW + BW + B]
            ps = w_in.T @ rhs1 + w_nx.T @ rhs2  # [128, 512]
            nat = _block_t(ps)  # y[:, base+5+k*BW : base+5+k*BW+BW]
            lo = base + HALF + k * BW
            hi = min(lo + BW, Lx - HALF)
            if hi > lo:
                out[:, lo:hi] = nat[:, : hi - lo]
        if n == 0:
            ps_e = w_ef.T @ btx[:, :B]  # [128, 32]
            nat_e = _block_t(ps_e)
            out[:, :HALF] = nat_e[:, :HALF]
        if n == n_chunks - 1:
            ps_e = w_el.T @ btx[:, ch - B : ch]
            nat_e = _block_t(ps_e)
            out[:, Lx - HALF :] = nat_e[:, :HALF]
    return out


# ---------------------------------------------------------------------------
# Bass kernel
# ---------------------------------------------------------------------------

_NC_CACHE = None


def _build_nc(l: int = L, ch: int = CH):
    import concourse.tile as tile
    from concourse import bacc, mybir

    assert l % ch == 0 and ch % BW == 0
    nc = bacc.Bacc(
        "TRN2",
        target_bir_lowering=False,
        debug=False,
        enable_asserts=False,
        num_devices=N_CORES,
    )
    f32 = mybir.dt.float32
    # fp16 on the matmul path: 1 cyc/col PE streaming (vs 4 for fp32),
    # 11 mantissa bits keep the filter error ~1e-3 relative.
    f16 = mybir.dt.float16

    x = nc.dram_tensor("x", [P, l], f32, kind="ExternalInput").ap()
    w_aps = {
        name: nc.dram_tensor(f"w_{name}", [P, P], f16, kind="ExternalInput").ap()
        for name in ("in", "nx", "ef", "el")
    }
    y = nc.dram_tensor("y", [P, l], f32, kind="ExternalOutput").ap()

    # Work units: (base, width). Main units at full `ch` (big DMAs sustain
    # ~380GB/s); the first and last `ch` columns run as half-width units to
    # shorten pipeline fill and drain.
    ce = min(CH_EDGE, ch)
    if l >= 2 * ch and ch == 2 * ce:
        units = [(b, ch) for b in range(0, l - ch, ch)]
        units += [(l - ch, ce), (l - ce, ce)]
    else:
        units = [(b, ch) for b in range(0, l, ch)]
    assert sum(w for _, w in units) == l
    n_units = len(units)

    with tile.TileContext(nc) as tc:
        with ExitStack() as ctx:
            consts = ctx.enter_context(tc.tile_pool(name="consts", bufs=1))
            in_pool = ctx.enter_context(tc.tile_pool(name="inp", bufs=4))
            cv_pool = ctx.enter_context(tc.tile_pool(name="cvp", bufs=4))
            bt_pool = ctx.enter_context(tc.tile_pool(name="btp", bufs=3))
            out_pool = ctx.enter_context(tc.tile_pool(name="outp", bufs=3))
            ps_pool = ctx.enter_context(tc.tile_pool(name="ps", bufs=3, space="PSUM"))
            ps_edge = ctx.enter_context(tc.tile_pool(name="pse", bufs=2, space="PSUM"))

            wt = {}

            def load_weights():
                for name, ap in w_aps.items():
                    t = consts.tile([P, P], f16, tag=f"w_{name}")
                    nc.sync.dma_start(t[:], ap)
                    wt[name] = t

            def load_cvt(n):
                """Input DMA + fp16 convert for unit n."""
                base, w = units[n]
                xin = in_pool.tile([P, ch + B], f32, tag="xin")
                valid = min(w + B, l - base)
                nc.sync.dma_start(xin[:, :valid], x[:, base : base + valid])
                if valid < w + B:
                    nc.gpsimd.memset(xin[:, valid : w + B], 0.0)
                xh = cv_pool.tile([P, ch + B], f16, tag="xh")
                nc.scalar.copy(xh[:, : w + B], xin[:, : w + B])
                return xh

            def tr_in(n, xh):
                """DVE 32x32 block transpose of a converted unit."""
                _, w = units[n]
                btx = bt_pool.tile([P, ch + B], f16, tag="btx")
                nc.vector.transpose(btx[:, : w + B], xh[:, : w + B])
                return btx

            # software pipeline: DMA+convert run 3 units ahead, the input
            # transpose 1 ahead, so no engine queue ever stalls at its head.
            xhs = {0: load_cvt(0)}
            load_weights()  # tiny; queued behind the first big input DMA
            for i in range(1, min(3, n_units)):
                xhs[i] = load_cvt(i)
            btxs = {0: tr_in(0, xhs.pop(0))}

            for n in range(n_units):
                base, w = units[n]
                first, last = n == 0, n == n_units - 1
                npt = w // PSW
                btx = btxs.pop(n)
                ystag = out_pool.tile([P, ch + HALF], f32, tag="ystag")
                off = HALF if first else 0

                pss = []
                for k in range(npt):
                    ps = ps_pool.tile([P, PSW], f32, tag="ps")
                    for h in range(PSW // BW):
                        c0 = k * PSW + h * BW
                        po = h * BW
                        nc.tensor.matmul(
                            ps[:, po : po + BW],
                            wt["in"][:],
                            btx[:, c0 : c0 + BW],
                            start=True,
                            stop=False,
                        )
                        nc.tensor.matmul(
                            ps[:, po : po + BW],
                            wt["nx"][:],
                            btx[:, c0 + B : c0 + BW + B],
                            start=False,
                            stop=True,
                        )
                    pss.append(ps)

                if first or last:
                    wname, rhs = (
                        ("ef", btx[:, :B]) if first else ("el", btx[:, w - B : w])
                    )
                    pse = ps_edge.tile([P, BW], f32, tag="edge")
                    nc.tensor.matmul(
                        pse[:, :B], wt[wname][:], rhs, start=True, stop=True
                    )

                # queue next unit's input work ahead of the psum drain
                if n + 3 < n_units:
                    xhs[n + 3] = load_cvt(n + 3)
                if n + 1 < n_units:
                    btxs[n + 1] = tr_in(n + 1, xhs.pop(n + 1))

                # drain psum to natural layout; big units stream out in two
                # half-DMAs, small (edge) units in one.
                # ystag col i holds y col ylo+i; valid cols [0, w_total).
                ylo = 0 if first else base + HALF
                w_total = w + HALF if first else (w - HALF if last else w)
                split = off + (npt // 2) * PSW if npt >= 4 else None
                if first:  # left edge cols y[0:5] -> ystag[0:5]
                    esb = out_pool.tile([P, B], f32, tag="esb")
                    nc.vector.transpose(esb[:], pse[:, :B])
                    nc.scalar.copy(ystag[:, :HALF], esb[:, :HALF])
                # on the final (small) units the DMA engines are draining idle:
                # stream out per psum tile to cut end-of-kernel latency
                tail_unit = split is None and n >= n_units - 2
                lo = 0
                for k in range(npt):
                    nc.vector.transpose(
                        ystag[:, off + k * PSW : off + (k + 1) * PSW], pss[k][:]
                    )
                    if split is not None and k == npt // 2 - 1:
                        nc.sync.dma_start(
                            y[:, ylo : ylo + split], ystag[:, :split]
                        )
                        lo = split
                    elif tail_unit and k < npt - 1:
                        hi = off + (k + 1) * PSW
                        nc.sync.dma_start(
                            y[:, ylo + lo : ylo + hi], ystag[:, lo:hi]
                        )
                        lo = hi
                if last:  # right edge cols y[l-5:l] -> ystag[w-10:w-5]
                    esb = out_pool.tile([P, B], f32, tag="esb")
                    nc.vector.transpose(esb[:], pse[:, :B])
                    ecol = w - 2 * HALF
                    nc.scalar.copy(ystag[:, ecol : ecol + HALF], esb[:, :HALF])
                nc.sync.dma_start(
                    y[:, ylo + lo : ylo + w_total], ystag[:, lo:w_total]
                )

    nc.compile()
    return nc


def _get_nc():
    global _NC_CACHE
    if _NC_CACHE is None:
        _NC_CACHE = _build_nc()
    return _NC_CACHE


def _in_maps(x: np.ndarray) -> list[dict]:
    w_in, w_nx, w_ef, w_el = (w.astype(np.float16) for w in _build_weights())
    xr = np.ascontiguousarray(x.reshape(ROWS_TOTAL, L))
    return [
        {
            "x": xr[i * ROWS_PER_CORE : (i + 1) * ROWS_PER_CORE],
            "w_in": w_in,
            "w_nx": w_nx,
            "w_ef": w_ef,
            "w_el": w_el,
        }
        for i in range(N_CORES)
    ]


def kernel(x: np.ndarray) -> np.ndarray:
    from concourse.bass_utils import run_bass_kernel_spmd

    assert x.shape == (64, 16, L) and x.dtype == np.float32
    nc = _get_nc()
    in_maps = _in_maps(x)
    res = run_bass_kernel_spmd(nc, in_maps, core_ids=list(range(N_CORES)))
    out = np.concatenate([r["y"] for r in res.results], axis=0)
    return out.reshape(64, 16, L)


if __name__ == "__main__":
    rng = np.random.default_rng(0)
    xt = rng.standard_normal((P, 8192)).astype(np.float32)
    ref = reference_rows(xt)
    sim = simulate_host(xt)
    err = np.abs(sim - ref).max()
    rel = err / np.abs(ref).max()
    print(f"host sim vs ref: max abs {err:.3e}  rel {rel:.3e}")
